# revision 1
# baseline (speedup 1.0000x reference)
"""Trainium2 Bass kernel for nn_DecoderUnit: additive attention + GRUCell +
vocab projection, data-parallel over batch B=256 across 8 NeuronCores.

Per core (B_LOC=32):
  xProj  = xT_b @ xe_w.T         (PE, fp16, [a,t] layout, 4x4 128-tiles per b)
  tanh(xProj + sProj + bias)     (ACT, bias per-partition, psum->sbuf)
  scoresT[t,b] = tanh.T @ we_w   (PE, N=1 matmuls, accumulated over a-chunks)
  softmax over t (groups of 16 b: PE transpose -> [b,t], DVE/ACT fused)
  contextT[xd,b] = x_b.T @ alphaT (PE, x streamed again in natural layout)
  GRU gates transposed [1536,b]  (PE + DVE/ACT, biases per-partition)
  logits = h @ fc_w.T + fc_b     (PE, fc_w.T streamed, fc_b via bcast DMA + DVE add)
"""
import functools
import numpy as np

import concourse.bass as bass
import concourse.mybir as mybir
import concourse.tile as tile
from concourse import bacc
from concourse.bass_utils import run_bass_kernel_spmd
from concourse.masks import make_identity

B, T, XD, SD, AD, YD = 256, 256, 512, 512, 512, 6625
NCORES = 8
BL = B // NCORES          # 32
G = 16                    # softmax group size (2 groups)
NYB = 13                  # ceil(6625/512) vocab blocks
YB = 512

f32 = mybir.dt.float32
f16 = mybir.dt.float16
i32 = mybir.dt.int32
OP = mybir.AluOpType
AF = mybir.ActivationFunctionType
AX = mybir.AxisListType


def _build():
    nc = bacc.Bacc(name="decoder_unit")

    # ---- per-core DRAM I/O ----
    xT_d = nc.dram_tensor("xT", [BL, XD, T], f16, kind="ExternalInput")
    xn_d = nc.dram_tensor("xn", [BL, T, XD], f16, kind="ExternalInput")
    sT_d = nc.dram_tensor("sT", [SD, BL], f32, kind="ExternalInput")
    yidx_d = nc.dram_tensor("yidx", [BL, 1], i32, kind="ExternalInput")
    xewT_d = nc.dram_tensor("xewT", [XD, AD], f16, kind="ExternalInput")
    sewT_d = nc.dram_tensor("sewT", [SD, AD], f16, kind="ExternalInput")
    wewT_d = nc.dram_tensor("wewT", [AD, 1], f16, kind="ExternalInput")
    xsb_d = nc.dram_tensor("xsb", [AD, 1], f32, kind="ExternalInput")
    emb_d = nc.dram_tensor("emb", [YD + 1, AD], f32, kind="ExternalInput")
    wihT_d = nc.dram_tensor("wihT", [AD + XD, 3 * SD], f16, kind="ExternalInput")
    whhT_d = nc.dram_tensor("whhT", [SD, 3 * SD], f16, kind="ExternalInput")
    bihT_d = nc.dram_tensor("bihT", [3 * SD, 1], f32, kind="ExternalInput")
    bhhT_d = nc.dram_tensor("bhhT", [3 * SD, 1], f32, kind="ExternalInput")
    fcwT_d = nc.dram_tensor("fcwT", [SD, YD], f16, kind="ExternalInput")
    fcb_d = nc.dram_tensor("fcb", [1, YD], f32, kind="ExternalInput")
    logits_d = nc.dram_tensor("logits", [BL, YD], f32, kind="ExternalOutput")
    h_d = nc.dram_tensor("h_out", [BL, SD], f32, kind="ExternalOutput")

    fcwT_ap = fcwT_d[:, :].rearrange("(c p) y -> p c y", p=128)  # [128,4,YD]

    with tile.TileContext(nc) as tc:
        with (
            tc.tile_pool(name="const", bufs=1) as C,
            tc.tile_pool(name="sm", bufs=2) as SM,
            tc.tile_pool(name="tp_ps", bufs=2, space="PSUM") as TP,
            tc.tile_pool(name="ctx_ps", bufs=1, space="PSUM") as CXP,
        ):
            # ================= phase 0: constants, sProj, yProj =============
            ident = C.tile([128, 128], f32)
            make_identity(nc, ident[:])

            sT_sb = C.tile([128, 4, BL], f32)
            nc.sync.dma_start(out=sT_sb[:], in_=sT_d[:, :].rearrange("(c p) b -> p c b", p=128))
            sT16 = C.tile([128, 4, BL], f16)
            nc.vector.tensor_copy(out=sT16[:], in_=sT_sb[:])

            xewT_sb = C.tile([128, 4, AD], f16)
            nc.sync.dma_start(out=xewT_sb[:], in_=xewT_d[:, :].rearrange("(c p) a -> p c a", p=128))
            sewT_sb = C.tile([128, 4, AD], f16)
            nc.sync.dma_start(out=sewT_sb[:], in_=sewT_d[:, :].rearrange("(c p) a -> p c a", p=128))
            wew_sb = C.tile([128, 4, 1], f16)
            nc.sync.dma_start(out=wew_sb[:], in_=wewT_d[:, :].rearrange("(c p) o -> p c o", p=128))
            xsb_sb = C.tile([128, 4, 1], f32)
            nc.sync.dma_start(out=xsb_sb[:], in_=xsb_d[:, :].rearrange("(c p) o -> p c o", p=128))

            # PE warmup so later fp32 transposes carry a single sem wait
            warm = TP.tile([128, 128], f32, tag="tp")
            nc.tensor.transpose(out=warm[:], in_=ident[:], identity=ident[:])

            # embedding gather
            idx_sb = C.tile([BL, 1], i32)
            nc.sync.dma_start(out=idx_sb[:], in_=yidx_d[:, :])
            yproj_sb = C.tile([BL, AD], f32)
            nc.gpsimd.indirect_dma_start(
                out=yproj_sb[:], out_offset=None, in_=emb_d[:, :],
                in_offset=bass.IndirectOffsetOnAxis(ap=idx_sb[:, :1], axis=0))
            yproj2 = C.tile([BL, AD], f32)
            nc.vector.tensor_copy(out=yproj2[:], in_=yproj_sb[:])

            ccT_sb = C.tile([128, 8, BL], f16)
            for c in range(4):
                ydT = TP.tile([128, BL], f32, tag="tp")
                nc.tensor.transpose(out=ydT[:], in_=yproj2[:, c * 128:(c + 1) * 128],
                                    identity=ident[:BL, :BL])
                nc.vector.tensor_copy(out=ccT_sb[:, c, :], in_=ydT[:])

            # sProjB[a, b] = se_w @ s.T + (xe_b + se_b)
            sProjB_sb = C.tile([128, 4, BL], f32)
            with tc.tile_pool(name="sproj_ps", bufs=1, space="PSUM") as SPP:
                sp_ps = SPP.tile([128, 4, BL], f32)
                for m in range(4):
                    for k in range(4):
                        nc.tensor.matmul(sp_ps[:, m, :],
                                         lhsT=sewT_sb[:, k, m * 128:(m + 1) * 128],
                                         rhs=sT16[:, k, :],
                                         start=(k == 0), stop=(k == 3))
                for m in range(4):
                    nc.vector.tensor_scalar(out=sProjB_sb[:, m, :], in0=sp_ps[:, m, :],
                                            scalar1=xsb_sb[:, m, :], scalar2=None,
                                            op0=OP.add)

            ctxT_ps = CXP.tile([128, 4, BL], f32)

            # ================= phase 1: attention (groups of G batches) =====
            with (
                tc.tile_pool(name="xt", bufs=3) as XT,
                tc.tile_pool(name="th", bufs=3) as TH,
                tc.tile_pool(name="xn", bufs=3) as XN,
                tc.tile_pool(name="xp_ps", bufs=2, space="PSUM") as XPP,
                tc.tile_pool(name="sc_ps", bufs=1, space="PSUM") as SCP,
            ):
                for g in range(BL // G):
                    scoresT_ps = SCP.tile([128, 2, G], f32)
                    for bl in range(G):
                        b = g * G + bl
                        xt_sb = XT.tile([128, 4, T], f16)
                        nc.sync.dma_start(
                            out=xt_sb[:],
                            in_=xT_d[b, :, :].rearrange("(c p) t -> p c t", p=128))
                        xp_ps = XPP.tile([128, 4, T], f32)
                        for m in range(4):
                            for k in range(4):
                                nc.tensor.matmul(xp_ps[:, m, :],
                                                 lhsT=xewT_sb[:, k, m * 128:(m + 1) * 128],
                                                 rhs=xt_sb[:, k, :],
                                                 start=(k == 0), stop=(k == 3))
                        th_sb = TH.tile([128, 4, T], f16)
                        for c in range(4):
                            nc.scalar.activation(out=th_sb[:, c, :], in_=xp_ps[:, c, :],
                                                 func=AF.Tanh,
                                                 bias=sProjB_sb[:, c, b:b + 1], scale=1.0)
                        for half in range(2):
                            for c in range(4):
                                nc.tensor.matmul(
                                    scoresT_ps[:, half, bl:bl + 1],
                                    lhsT=th_sb[:, c, half * 128:(half + 1) * 128],
                                    rhs=wew_sb[:, c, :],
                                    start=(c == 0), stop=(c == 3))

                    # --- group softmax: scoresT [t, G] -> alpha [G, t] -> alphaT
                    scoresT_sb = SM.tile([128, 2, G], f32)
                    nc.vector.tensor_copy(out=scoresT_sb[:], in_=scoresT_ps[:])
                    scores_sb = SM.tile([G, T], f32)
                    for half in range(2):
                        scT = TP.tile([G, 128], f32, tag="tp")
                        nc.tensor.transpose(out=scT[:], in_=scoresT_sb[:, half, :],
                                            identity=ident[:])
                        nc.vector.tensor_copy(out=scores_sb[:, half * 128:(half + 1) * 128],
                                              in_=scT[:])
                    nmx = SM.tile([G, 1], f32)
                    nc.vector.tensor_reduce(out=nmx[:], in_=scores_sb[:], axis=AX.X,
                                            op=OP.max, negate=True)
                    pr = SM.tile([G, T], f32)
                    sume = SM.tile([G, 1], f32)
                    nc.scalar.activation(out=pr[:], in_=scores_sb[:], func=AF.Exp,
                                         bias=nmx[:], scale=1.0, accum_out=sume[:])
                    rsum = SM.tile([G, 1], f32)
                    nc.vector.reciprocal(out=rsum[:], in_=sume[:])
                    alpha = SM.tile([G, T], f32)
                    nc.vector.tensor_scalar_mul(out=alpha[:], in0=pr[:], scalar1=rsum[:])
                    alphaT_sb = SM.tile([128, 2, G], f16)
                    for half in range(2):
                        alT = TP.tile([128, G], f32, tag="tp")
                        nc.tensor.transpose(out=alT[:], in_=alpha[:, half * 128:(half + 1) * 128],
                                            identity=ident[:G, :G])
                        nc.vector.tensor_copy(out=alphaT_sb[:, half, :], in_=alT[:])

                    # --- context: ctxT[xd, b] = sum_t x[t, xd] * alpha[t]
                    for bl in range(G):
                        b = g * G + bl
                        xn_sb = XN.tile([128, 2, XD], f16)
                        nc.sync.dma_start(
                            out=xn_sb[:],
                            in_=xn_d[b, :, :].rearrange("(h p) x -> p h x", p=128))
                        for c in range(4):
                            for half in range(2):
                                nc.tensor.matmul(
                                    ctxT_ps[:, c, b:b + 1],
                                    lhsT=xn_sb[:, half, c * 128:(c + 1) * 128],
                                    rhs=alphaT_sb[:, half, bl:bl + 1],
                                    start=(half == 0), stop=(half == 1))

            for c in range(4):
                nc.vector.tensor_copy(out=ccT_sb[:, 4 + c, :], in_=ctxT_ps[:, c, :])

            # ================= phase 2: GRU + fc ===========================
            wihT_sb = C.tile([128, 8, 3 * SD], f16)
            nc.sync.dma_start(out=wihT_sb[:], in_=wihT_d[:, :].rearrange("(c p) n -> p c n", p=128))
            whhT_sb = C.tile([128, 4, 3 * SD], f16)
            nc.sync.dma_start(out=whhT_sb[:], in_=whhT_d[:, :].rearrange("(c p) n -> p c n", p=128))
            bihT_sb = C.tile([128, 12, 1], f32)
            nc.sync.dma_start(out=bihT_sb[:], in_=bihT_d[:, :].rearrange("(c p) o -> p c o", p=128))
            bhhT_sb = C.tile([128, 12, 1], f32)
            nc.sync.dma_start(out=bhhT_sb[:], in_=bhhT_d[:, :].rearrange("(c p) o -> p c o", p=128))

            with (
                tc.tile_pool(name="gru_ps", bufs=1, space="PSUM") as GRP,
                tc.tile_pool(name="lg_ps", bufs=3, space="PSUM") as LGP,
                tc.tile_pool(name="fcw", bufs=8) as FW,
            ):
                giT_ps = GRP.tile([128, 12, BL], f32)
                for m in range(12):
                    for k in range(8):
                        nc.tensor.matmul(giT_ps[:, m, :],
                                         lhsT=wihT_sb[:, k, m * 128:(m + 1) * 128],
                                         rhs=ccT_sb[:, k, :],
                                         start=(k == 0), stop=(k == 7))
                ghT_ps = GRP.tile([128, 12, BL], f32)
                for m in range(12):
                    for k in range(4):
                        nc.tensor.matmul(ghT_ps[:, m, :],
                                         lhsT=whhT_sb[:, k, m * 128:(m + 1) * 128],
                                         rhs=sT16[:, k, :],
                                         start=(k == 0), stop=(k == 3))
                # ghb = gh + b_hh (needed pre-multiplied by r for n gate)
                ghb_sb = C.tile([128, 12, BL], f32)
                for m in range(12):
                    nc.vector.tensor_scalar(out=ghb_sb[:, m, :], in0=ghT_ps[:, m, :],
                                            scalar1=bhhT_sb[:, m, :], scalar2=None,
                                            op0=OP.add)
                rT = C.tile([128, 4, BL], f32)
                zT = C.tile([128, 4, BL], f32)
                nT = C.tile([128, 4, BL], f32)
                sum_sb = C.tile([128, 4, BL], f32)
                sum2_sb = C.tile([128, 4, BL], f32)
                tmp_sb = C.tile([128, 4, BL], f32)
                for c in range(4):
                    nc.vector.tensor_add(out=sum_sb[:, c, :], in0=giT_ps[:, c, :],
                                         in1=ghb_sb[:, c, :])
                    nc.scalar.activation(out=rT[:, c, :], in_=sum_sb[:, c, :],
                                         func=AF.Sigmoid, bias=bihT_sb[:, c, :], scale=1.0)
                for c in range(4):
                    nc.vector.tensor_add(out=sum_sb[:, c, :], in0=giT_ps[:, 4 + c, :],
                                         in1=ghb_sb[:, 4 + c, :])
                    nc.scalar.activation(out=zT[:, c, :], in_=sum_sb[:, c, :],
                                         func=AF.Sigmoid, bias=bihT_sb[:, 4 + c, :], scale=1.0)
                for c in range(4):
                    nc.vector.tensor_mul(out=tmp_sb[:, c, :], in0=rT[:, c, :],
                                         in1=ghb_sb[:, 8 + c, :])
                    nc.vector.tensor_add(out=sum2_sb[:, c, :], in0=giT_ps[:, 8 + c, :],
                                         in1=tmp_sb[:, c, :])
                    nc.scalar.activation(out=nT[:, c, :], in_=sum2_sb[:, c, :],
                                         func=AF.Tanh, bias=bihT_sb[:, 8 + c, :], scale=1.0)
                # h = n + z * (s - n)
                hT_sb = C.tile([128, 4, BL], f32)
                d_sb = C.tile([128, 4, BL], f32)
                nc.vector.tensor_sub(out=d_sb[:], in0=sT_sb[:], in1=nT[:])
                nc.vector.tensor_mul(out=d_sb[:], in0=zT[:], in1=d_sb[:])
                nc.vector.tensor_add(out=hT_sb[:], in0=nT[:], in1=d_sb[:])
                hT16 = C.tile([128, 4, BL], f16)
                nc.vector.tensor_copy(out=hT16[:], in_=hT_sb[:])

                # h output (transpose back to [b, sd])
                h_sb = C.tile([BL, SD], f32)
                for c in range(4):
                    hp = TP.tile([BL, 128], f32, tag="tp")
                    nc.tensor.transpose(out=hp[:], in_=hT_sb[:, c, :], identity=ident[:])
                    nc.vector.tensor_copy(out=h_sb[:, c * 128:(c + 1) * 128], in_=hp[:])
                nc.sync.dma_start(out=h_d[:, :], in_=h_sb[:])

                # fc: logits[b, y] = h @ fc_w.T + fc_b
                fcb_bc = C.tile([BL, YD], f32)
                nc.gpsimd.dma_start(out=fcb_bc[:], in_=fcb_d[0:1, :].to_broadcast([BL, YD]))
                logits_sb = C.tile([BL, YD], f32)
                for nb in range(NYB):
                    y0 = nb * YB
                    yw = min(YB, YD - y0)
                    fcw_sb = FW.tile([128, 4, YB], f16)
                    nc.sync.dma_start(out=fcw_sb[:, :, :yw], in_=fcwT_ap[:, :, y0:y0 + yw])
                    lg_ps = LGP.tile([BL, YB], f32)
                    for k in range(4):
                        nc.tensor.matmul(lg_ps[:, :yw], lhsT=hT16[:, k, :],
                                         rhs=fcw_sb[:, k, :yw],
                                         start=(k == 0), stop=(k == 3))
                    nc.vector.tensor_add(out=logits_sb[:, y0:y0 + yw], in0=lg_ps[:, :yw],
                                         in1=fcb_bc[:, y0:y0 + yw])
                nc.sync.dma_start(out=logits_d[:, :], in_=logits_sb[:])

    nc.compile()
    return nc


@functools.lru_cache(maxsize=1)
def _program():
    return _build()


def _prep_inputs(inputs):
    x = np.asarray(inputs["x"], np.float32)
    s = np.asarray(inputs["sPrev"], np.float32)[0]          # [B, SD]
    y = np.asarray(inputs["yPrev"]).astype(np.int32)

    x4 = x.reshape(NCORES, BL, T, XD)
    xT = np.ascontiguousarray(x4.transpose(0, 1, 3, 2)).astype(np.float16)
    xn = x4.astype(np.float16)
    sT = np.ascontiguousarray(s.reshape(NCORES, BL, SD).transpose(0, 2, 1))
    yidx = y.reshape(NCORES, BL, 1)

    xe_w = np.asarray(inputs["xe_w"], np.float32)
    se_w = np.asarray(inputs["se_w"], np.float32)
    we_w = np.asarray(inputs["we_w"], np.float32)
    xsb = (np.asarray(inputs["xe_b"], np.float32)
           + np.asarray(inputs["se_b"], np.float32)).reshape(AD, 1)
    emb = np.asarray(inputs["emb"], np.float32)
    wih = np.asarray(inputs["gru_w_ih"], np.float32)
    whh = np.asarray(inputs["gru_w_hh"], np.float32)
    bih = np.asarray(inputs["gru_b_ih"], np.float32).reshape(3 * SD, 1)
    bhh = np.asarray(inputs["gru_b_hh"], np.float32).reshape(3 * SD, 1)
    fcw = np.asarray(inputs["fc_w"], np.float32)
    fcb = np.asarray(inputs["fc_b"], np.float32).reshape(1, YD)

    shared = {
        "xewT": np.ascontiguousarray(xe_w.T).astype(np.float16),
        "sewT": np.ascontiguousarray(se_w.T).astype(np.float16),
        "wewT": np.ascontiguousarray(we_w.reshape(1, AD).T).astype(np.float16),
        "xsb": xsb,
        "emb": emb,
        "wihT": np.ascontiguousarray(wih.T).astype(np.float16),
        "whhT": np.ascontiguousarray(whh.T).astype(np.float16),
        "bihT": bih,
        "bhhT": bhh,
        "fcwT": np.ascontiguousarray(fcw.T).astype(np.float16),
        "fcb": fcb,
    }
    in_maps = []
    for c in range(NCORES):
        m = dict(shared)
        m["xT"] = xT[c]
        m["xn"] = xn[c]
        m["sT"] = sT[c]
        m["yidx"] = yidx[c]
        in_maps.append(m)
    return in_maps


def kernel(**inputs):
    nc = _program()
    in_maps = _prep_inputs(inputs)
    res = run_bass_kernel_spmd(nc, in_maps, core_ids=list(range(NCORES)))
    logits = np.concatenate([r["logits"] for r in res.results], axis=0)
    h = np.concatenate([r["h_out"] for r in res.results], axis=0)
    return logits, h
